# revision 38
# baseline (speedup 1.0000x reference)
"""Trainium2 Bass kernel for HTM spatial-pooler overlap + global top-k inhibition.

Problem (nn_HTMModel_19834158973432):
    overlap  = connections @ input_vector          # [4096] = [4096, 32768] @ [32768]
    boosted  = overlap * boosting_factors          # [4096]
    winners  = top_k(boosted, 82)                  # ties broken by lower index
    active   = one_hot(winners)                    # [4096] 0/1 mask
    returns (active, active * boosted)

Strategy (8 NeuronCores, SPMD):
  - connections/input_vector are binary 0/1, so an fp8(e4m3) cast is EXACT.
    Host pre-transposes each core's row shard [512, 32768] into a
    partition-major fp8 layout so the TensorEngine does multiply+accumulate
    in one pass, using DoubleRow fp8 matmuls (2 contractions of K=128 per
    instruction, 2x streaming throughput):
        psum[1, 512] += sum_slot vt2[:, slot, n].T @ conn_pair[:, slot, :]
    DMA traffic is 16 MiB/core (4x less than f32, ~DMA roofline bound);
    conn chunks stream on 3 DMA rings with small first chunks, and ~20
    warmup matmuls keep the PE_HAM clock warm during the fill.
  - boosted = overlap * boost fits fp16 EXACTLY (integers <= 2048); the
    AllGather carries 1 KB/rank of fp16.  A tiny dummy collective at t=0
    absorbs the CC-stream bootstrap during the matvec.
  - Top-82 without sorting: bucket edges are whole boosted values, so
      gp[e] = #{c : boosted[c] >= bmax-127+e}
    is one elementwise pass over a PE-broadcast of the gathered boosted
    vector, split DVE (is_ge halves) / ACT (sign(b-edge+.5) halves), giving
    b82 (boosted value of the 82nd winner) and n_hi (# strictly above);
    the columns with boosted == b82 are then selected by global index rank
    (triangular-matmul prefix across partitions + Kogge-Stone along free).
  - Each core writes the full [2, 4096] output; the host returns core 0's.
"""

import sys

if "/opt/trn_rl_repo" not in sys.path:
    sys.path.insert(0, "/opt/trn_rl_repo")

import numpy as np

C_TOT = 4096          # minicolumns
IN = 32768            # input size
CORES = 8
ROWS = C_TOT // CORES  # 512 rows per core
K_ACT = 82            # active columns per inhibition area

NCH = IN // 128        # 256 i-chunks of 128 (contraction per matmul slot)
NPAIR = NCH // 2       # 128 DoubleRow matmuls

B_MARGIN = 127        # bucket-search window below bmax (in boosted units)
EARLY_CC_WARM = True  # tiny dummy collective at t=0 absorbs CC bootstrap


def _build_nc(stage=4):
    # stage: 1=matvec only, 2=+allgather, 4=full
    from concourse import bacc, mybir, tile

    f32 = mybir.dt.float32
    f16 = mybir.dt.float16
    fp8 = mybir.dt.float8e4
    Alu = mybir.AluOpType
    Act = mybir.ActivationFunctionType
    DR = mybir.MatmulPerfMode.DoubleRow

    nc = bacc.Bacc("TRN2", target_bir_lowering=False, debug=False,
                   enable_asserts=False, num_devices=CORES)

    conn8 = nc.dram_tensor("conn8", [128, NCH * ROWS], fp8, kind="ExternalInput")
    vt2 = nc.dram_tensor("vt2", [128, NCH], fp8, kind="ExternalInput")
    boostl = nc.dram_tensor("boostl", [ROWS], f32, kind="ExternalInput")
    consts16 = nc.dram_tensor("consts16", [128, 256], f16, kind="ExternalInput")
    out = nc.dram_tensor("out", [2, C_TOT], f32, kind="ExternalOutput")

    with tile.TileContext(nc) as tc:
        with (
            tc.tile_pool(name="const", bufs=1) as constp,
            tc.tile_pool(name="cpool", bufs=1) as cpool,
            tc.tile_pool(name="scrp", bufs=2) as scrp,
            tc.tile_pool(name="dramp", bufs=1, space="DRAM") as dramp,
            tc.tile_pool(name="ovp", bufs=1, space="PSUM") as ovp,
            tc.tile_pool(name="warmp", bufs=1, space="PSUM") as warmp,
            tc.tile_pool(name="bcp", bufs=4, space="PSUM") as bcp,
            tc.tile_pool(name="sps", bufs=2, space="PSUM") as sps,
        ):
            # ---- tiny dummy collective first: forces the CC-stream rank
            # handshake to run during the matvec so the real AllGather is
            # not serialized behind a cold bootstrap ----
            if EARLY_CC_WARM and stage >= 2:
                wsrc16 = constp.tile([1, 8], f16, name="wsrc16")
                nc.vector.memset(wsrc16[:], 0.0)
                ccw_in = dramp.tile([8], f16, name="ccw_in")
                nc.gpsimd.dma_start(ccw_in.rearrange("(a f) -> a f", a=1),
                                    wsrc16[:])
                ccw_out = dramp.tile([8 * CORES], f16, name="ccw_out",
                                     addr_space="Shared")
                nc.gpsimd.collective_compute(
                    "AllGather", Alu.bypass,
                    replica_groups=[list(range(CORES))],
                    ins=[ccw_in.opt()], outs=[ccw_out.opt()])

            # ---- matvec input DMAs first: vt2 then the conn chunks.
            # Ascending chunk sizes (in DoubleRow pairs): small first chunks
            # land fast so the MM stream starts early; bandwidth amortizes
            # over the 1 MiB steady-state chunks.
            vt_sb = constp.tile([128, NCH], fp8, name="vt_sb")
            nc.sync.dma_start(vt_sb[:], vt2.ap())
            chunk_pairs = [2, 2, 4, 4] + [8] * 14 + [4]
            assert sum(chunk_pairs) == NPAIR
            cts = []
            off = 0
            engs = [nc.sync, nc.scalar, nc.sync, nc.scalar, nc.gpsimd]
            for k, cp_n in enumerate(chunk_pairs):
                w = cp_n * 2 * ROWS
                ct = cpool.tile([128, w], fp8, name=f"ct_{k}", tag=f"ct{k}")
                # 3 DMA rings: sync/scalar HWDGE take the latency-critical
                # early chunks, gpsimd's SWDGE row adds steady-state bandwidth
                eng = engs[k % len(engs)]
                eng.dma_start(ct[:], conn8.ap()[:, off:off + w])
                cts.append(ct)
                off += w

            # ---- constants (issued on gpsimd, off the critical path) ----
            cs16 = constp.tile([128, 256], f16, name="cs16")
            nc.gpsimd.dma_start(cs16[:], consts16.ap())
            ident16 = cs16[:, 0:128]
            tri16 = cs16[:, 128:256]
            boost_sb = constp.tile([1, ROWS], f32, name="boost_sb")
            nc.gpsimd.dma_start(boost_sb[:], boostl.ap()[None, :])
            ones_row16 = constp.tile([1, 128], f16, name="ones_row16")
            nc.vector.memset(ones_row16[:], 1.0)
            ones_col16 = constp.tile([128, 1], f16, name="ones_col16")
            nc.vector.memset(ones_col16[:], 1.0)
            ramp = constp.tile([128, 1], f32, name="ramp")
            nc.gpsimd.iota(ramp[:], pattern=[[0, 1]], base=0,
                           channel_multiplier=1,
                           allow_small_or_imprecise_dtypes=True)

            # ---- PE warmup: ~20 throwaway matmuls during the DMA fill keep
            # the PE_HAM activity window busy so the real MM stream runs at
            # 2.4 GHz (warm) instead of 1.2 GHz (cold). ----
            wrow16 = constp.tile([1, 512], f16, name="wrow16")
            nc.vector.memset(wrow16[:], 0.0)
            warm_ps = warmp.tile([128, 512], f32, name="warm_ps")
            for w in range(14):
                nc.tensor.matmul(warm_ps[:], lhsT=ones_row16[:], rhs=wrow16[:],
                                 start=True, stop=True)

            # ---- matvec: 128 DoubleRow fp8 matmuls accumulate into PSUM ----
            ov_ps = ovp.tile([1, ROWS], f32, name="ov_ps", tag="ov")
            vt_pairs = vt_sb.rearrange("p (two n) -> p two n", two=2)
            pr = 0
            for k, cp_n in enumerate(chunk_pairs):
                ctp = cts[k].rearrange("p (j two n) -> p j two n", j=cp_n,
                                       two=2)
                for j in range(cp_n):
                    nc.tensor.matmul(
                        ov_ps[:],
                        lhsT=vt_pairs[:, :, pr:pr + 1],
                        rhs=ctp[:, j],
                        start=(pr == 0), stop=(pr == NPAIR - 1),
                        perf_mode=DR,
                    )
                    pr += 1
            assert pr == NPAIR

            if stage <= 1:
                nc.sync.dma_start(out.ap()[0][0:ROWS][None, :], ov_ps[:])
                nc.compile()
                return nc

            # ---- boosted (fp16-exact) -> AllGather 1 KB/rank ----
            bl16 = constp.tile([1, ROWS], f16, name="bl16")
            nc.vector.tensor_tensor(bl16[:], ov_ps[:], boost_sb[:], Alu.mult)
            cc_in = dramp.tile([ROWS], f16, name="cc_in")
            cc_out = dramp.tile([C_TOT], f16, name="cc_out",
                                addr_space="Shared")
            nc.sync.dma_start(cc_in.rearrange("(a f) -> a f", a=1), bl16[:])
            nc.gpsimd.collective_compute(
                "AllGather", Alu.bypass,
                replica_groups=[list(range(CORES))],
                ins=[cc_in.opt()], outs=[cc_out.opt()])
            # keep the PE_HAM busy across the AllGather wait so the tail
            # matmuls run warm (2.4 GHz)
            if stage >= 3:
                for w in range(26):
                    nc.tensor.matmul(warm_ps[:], lhsT=ones_row16[:],
                                     rhs=wrow16[:], start=True, stop=True)

            if stage == 2:
                t16 = constp.tile([128, 32], f16, name="t16")
                nc.sync.dma_start(t16[:],
                                  cc_out.rearrange("(p f) -> p f", p=128))
                t32 = constp.tile([128, 32], f32, name="t32")
                nc.vector.tensor_copy(t32[:], t16[:])
                nc.sync.dma_start(
                    out.ap()[0].rearrange("(p f) -> p f", p=128), t32[:])
                nc.compile()
                return nc

            # ---- gathered boosted in two layouts (concurrent DMAs) ----
            b32_16 = constp.tile([128, 32], f16, name="b32_16")
            nc.sync.dma_start(b32_16[:], cc_out.rearrange("(p f) -> p f", p=128))
            brow16 = constp.tile([1, C_TOT], f16, name="brow16")
            nc.scalar.dma_start(brow16[:], cc_out.rearrange("(a f) -> a f", a=1))

            # ---- bmax (cross-partition max via fp16 PE transpose) ----
            bm16 = constp.tile([128, 1], f16, name="bm16")
            with nc.allow_low_precision(reason="max is fp16-exact"):
                nc.vector.reduce_max(bm16[:], b32_16[:],
                                     axis=mybir.AxisListType.X)
            bm_row = sps.tile([1, 128], f32, name="bm_row", tag="sps")
            nc.tensor.matmul(bm_row[:], lhsT=bm16[:], rhs=ident16,
                             start=True, stop=True)
            bmax16 = constp.tile([1, 1], f16, name="bmax16")
            with nc.allow_low_precision(reason="max is fp16-exact"):
                nc.vector.reduce_max(bmax16[:], bm_row[:],
                                     axis=mybir.AxisListType.X)
            bmax_ps = sps.tile([128, 1], f32, name="bmax_ps", tag="sps")
            nc.tensor.matmul(bmax_ps[:], lhsT=ones_row16[:], rhs=bmax16[:],
                             start=True, stop=True)
            # edges[p] = bmax - 127 + p ; nedge[p] = -edges[p] + 0.5
            b127 = constp.tile([128, 1], f32, name="b127")
            nc.vector.tensor_scalar(
                out=b127[:], in0=bmax_ps[:], scalar1=-float(B_MARGIN),
                scalar2=None, op0=Alu.add)
            edges = constp.tile([128, 1], f32, name="edges")
            nc.vector.tensor_tensor(edges[:], ramp[:], b127[:], Alu.add)
            bneg = constp.tile([128, 1], f32, name="bneg")
            nc.vector.tensor_scalar(
                out=bneg[:], in0=bmax_ps[:], scalar1=-1.0,
                scalar2=float(B_MARGIN) + 0.5, op0=Alu.mult, op1=Alu.add)
            nedge = constp.tile([128, 1], f32, name="nedge")
            nc.vector.tensor_scalar(
                out=nedge[:], in0=ramp[:], scalar1=-1.0, scalar2=bneg[:],
                op0=Alu.mult, op1=Alu.add)
            boosted32 = constp.tile([128, 32], f32, name="boosted32")
            nc.vector.tensor_copy(boosted32[:], b32_16[:])

            # ---- PE-broadcast boosted to 128 partitions (quarters), then
            # count cnt_ge(edge_p): quarters 0,1 on DVE is_ge, 2,3 on ACT
            # via sum of sign(b - edge + 0.5) ----
            # eighth-sized broadcast tiles: 8 PE matmuls (N=512) feed the
            # two count engines round-robin (evens on DVE is_ge, odds on ACT
            # Sign); bufs=4 PSUM quarters avoid reuse stalls
            gph = {}
            for h in (0, 1, 2, 3, 4, 5, 6, 7):
                ps = bcp.tile([128, 512], f32, name=f"kbc{h}", tag="kbc")
                nc.tensor.matmul(
                    ps[:], lhsT=ones_row16[:],
                    rhs=brow16[:, h * 512:(h + 1) * 512],
                    start=True, stop=True)
                scr = scrp.tile([128, 512], f16, name=f"cmp{h}",
                                tag="cmp", bufs=4)
                g = constp.tile([128, 1], f32, name=f"gph{h}")
                if h % 2 == 0:
                    nc.vector.tensor_scalar(
                        out=scr[:], in0=ps[:], scalar1=edges[:], scalar2=None,
                        op0=Alu.is_ge, op1=Alu.add, accum_out=g[:])
                else:
                    nc.scalar.activation(scr[:], ps[:], Act.Sign,
                                         bias=nedge[:], scale=1.0,
                                         accum_out=g[:])
                gph[h] = g
            gpa1 = constp.tile([128, 1], f32, name="gpa1")
            nc.vector.tensor_tensor(gpa1[:], gph[0][:], gph[2][:], Alu.add)
            gpa2 = constp.tile([128, 1], f32, name="gpa2")
            nc.vector.tensor_tensor(gpa2[:], gph[4][:], gph[6][:], Alu.add)
            gp01 = constp.tile([128, 1], f32, name="gp01")
            nc.vector.tensor_tensor(gp01[:], gpa1[:], gpa2[:], Alu.add)
            gpb1 = constp.tile([128, 1], f32, name="gpb1")
            nc.vector.tensor_tensor(gpb1[:], gph[1][:], gph[3][:], Alu.add)
            gpb2 = constp.tile([128, 1], f32, name="gpb2")
            nc.vector.tensor_tensor(gpb2[:], gph[5][:], gph[7][:], Alu.add)
            gp23 = constp.tile([128, 1], f32, name="gp23")
            nc.vector.tensor_tensor(gp23[:], gpb1[:], gpb2[:], Alu.add)
            # sign sums: g = 2*cnt - 512 per eighth -> cnt_odds = g/2 + 1024
            gp23s = constp.tile([128, 1], f32, name="gp23s")
            nc.vector.tensor_scalar(
                out=gp23s[:], in0=gp23[:], scalar1=0.5, scalar2=1024.0,
                op0=Alu.mult, op1=Alu.add)
            gp = constp.tile([128, 1], f32, name="gp")
            nc.vector.tensor_tensor(gp[:], gp01[:], gp23s[:], Alu.add)

            # cnt = #edges with cnt_ge >= 82  ->  b82 = bmax - 128 + cnt
            sel16 = constp.tile([128, 1], f16, name="sel16")
            nc.vector.tensor_scalar(
                out=sel16[:], in0=gp[:], scalar1=float(K_ACT), scalar2=None,
                op0=Alu.is_ge)
            cnt_ps = sps.tile([1, 1], f32, name="cnt_ps", tag="sps")
            nc.tensor.matmul(cnt_ps[:], lhsT=sel16[:], rhs=ones_col16[:],
                             start=True, stop=True)
            # n_hi = max over edges of gp*(gp<82)  (= cnt above b82's bucket)
            lt = constp.tile([128, 1], f32, name="lt")
            nc.vector.tensor_scalar(
                out=lt[:], in0=gp[:], scalar1=float(K_ACT), scalar2=None,
                op0=Alu.is_lt)
            gpm16 = constp.tile([128, 1], f16, name="gpm16")
            nc.vector.tensor_tensor(gpm16[:], gp[:], lt[:], Alu.mult)
            nhi_row = sps.tile([1, 128], f32, name="nhi_row", tag="sps")
            nc.tensor.matmul(nhi_row[:], lhsT=gpm16[:], rhs=ident16,
                             start=True, stop=True)
            nhi1 = constp.tile([1, 1], f32, name="nhi1")
            nc.vector.reduce_max(nhi1[:], nhi_row[:], axis=mybir.AxisListType.X)

            # pair16 = [b82, m]: b82 = (bmax - 128) + cnt ; m = 82 - n_hi
            bshift = constp.tile([1, 1], f32, name="bshift")
            nc.vector.tensor_scalar(
                out=bshift[:], in0=bmax16[:], scalar1=-128.0, scalar2=None,
                op0=Alu.add)
            pair16 = constp.tile([1, 2], f16, name="pair16")
            nc.vector.tensor_tensor(pair16[:, 0:1], cnt_ps[:], bshift[:],
                                    Alu.add)
            nc.vector.tensor_scalar(
                out=pair16[:, 1:2], in0=nhi1[:], scalar1=-1.0,
                scalar2=float(K_ACT), op0=Alu.mult, op1=Alu.add)
            pair_ps = sps.tile([128, 2], f32, name="pair_ps", tag="sps")
            nc.tensor.matmul(pair_ps[:], lhsT=ones_row16[:], rhs=pair16[:],
                             start=True, stop=True)
            pair_bc = constp.tile([128, 2], f32, name="pair_bc")
            nc.scalar.activation(pair_bc[:], pair_ps[:], Act.Copy)

            # ---- masks + global rank of equals ----
            gt32 = constp.tile([128, 32], f32, name="gt32")
            nc.vector.tensor_scalar(
                out=gt32[:], in0=boosted32[:], scalar1=pair_bc[:, 0:1],
                scalar2=None, op0=Alu.is_gt)
            eq32 = constp.tile([128, 32], f32, name="eq32")
            nc.vector.tensor_scalar(
                out=eq32[:], in0=boosted32[:], scalar1=pair_bc[:, 0:1],
                scalar2=None, op0=Alu.is_equal)
            # exclusive prefix count of eq across global c order:
            # per-partition totals -> strictly-lower-triangular matmul.
            # s16 summed on ACT so it overlaps the Kogge-Stone on DVE.
            s16 = constp.tile([128, 1], f16, name="s16")
            s_scr = constp.tile([128, 32], f16, name="s_scr")
            with nc.allow_low_precision(reason="sum of <=32 ones, fp16-exact"):
                nc.scalar.activation(s_scr[:], eq32[:], Act.Copy,
                                     accum_out=s16[:])
            P_ps = sps.tile([128, 1], f32, name="P_ps", tag="sps")
            nc.tensor.matmul(P_ps[:], lhsT=tri16, rhs=s16[:],
                             start=True, stop=True)
            P_sb = constp.tile([128, 1], f32, name="P_sb")
            nc.scalar.activation(P_sb[:], P_ps[:], Act.Copy)
            # within-partition inclusive prefix via Kogge-Stone on padded rows
            pada = constp.tile([128, 48], f32, name="pada")
            padb = constp.tile([128, 48], f32, name="padb")
            nc.vector.memset(pada[:, 0:16], 0.0)
            nc.vector.memset(padb[:, 0:16], 0.0)
            nc.vector.tensor_copy(pada[:, 16:48], eq32[:])
            src, dst = pada, padb
            for sh in (1, 2, 4, 8, 16):
                # pads [0:16] stay zero in both buffers (never written)
                nc.vector.tensor_tensor(
                    dst[:, 16:48], src[:, 16:48], src[:, 16 - sh:48 - sh],
                    Alu.add)
                src, dst = dst, src
            incl = src  # inclusive prefix in [:, 16:48]
            # rank = P[p] + incl - eq  (exclusive global prefix)
            rank32 = constp.tile([128, 32], f32, name="rank32")
            nc.vector.tensor_tensor(rank32[:], incl[:, 16:48], eq32[:],
                                    Alu.subtract)
            nc.vector.tensor_scalar(
                out=rank32[:], in0=rank32[:], scalar1=P_sb[:], scalar2=None,
                op0=Alu.add)
            # tie-selected = eq & (rank < m)
            tie32 = constp.tile([128, 32], f32, name="tie32")
            nc.vector.tensor_scalar(
                out=tie32[:], in0=rank32[:], scalar1=pair_bc[:, 1:2],
                scalar2=None, op0=Alu.is_lt)
            nc.vector.tensor_tensor(tie32[:], tie32[:], eq32[:], Alu.mult)

            # active|masked side by side in one tile -> single output DMA
            am = constp.tile([128, 64], f32, name="am")
            active32 = am[:, 0:32]
            nc.vector.tensor_tensor(active32, gt32[:], tie32[:], Alu.add)
            nc.vector.tensor_tensor(am[:, 32:64], active32, boosted32[:],
                                    Alu.mult)
            nc.sync.dma_start(
                out.ap().rearrange("r (p f) -> p r f", p=128),
                am.rearrange("p (r f) -> p r f", r=2))

    nc.compile()
    return nc


FP8_ONE = 0x38  # ml_dtypes.float8_e4m3(1.0).view(uint8)


def _make_in_maps(input_vector, connections, boosting_factors):
    import ml_dtypes

    fp8 = ml_dtypes.float8_e4m3
    v = np.asarray(input_vector, dtype=np.float32)
    c = np.asarray(connections, dtype=np.float32)
    b = np.ascontiguousarray(np.asarray(boosting_factors, dtype=np.float32))

    # consts16: [ident | strictly-lower-tri (tri[q,p]=1 iff q<p)]
    consts = np.zeros((128, 256), dtype=np.float16)
    consts[:, 0:128] = np.eye(128, dtype=np.float16)
    consts[:, 128:256] = np.triu(np.ones((128, 128), dtype=np.float16), k=1)

    # vt2[p, n] = v[(2n)*128 + p] for n<128, v[(2n+1)*128 + p] for n>=128
    vt = (v.reshape(NCH, 128).T != 0) * np.uint8(FP8_ONE)  # [128, 256]
    vt2 = np.ascontiguousarray(
        np.concatenate([vt[:, 0::2], vt[:, 1::2]], axis=1)).view(fp8)

    c8 = (c != 0) * np.uint8(FP8_ONE)  # [4096, 32768] uint8
    in_maps = []
    for r in range(CORES):
        # conn8[p, n*512 + j] = conn[r*512 + j, n*128 + p]
        shard = c8[r * ROWS:(r + 1) * ROWS]            # [512, 32768]
        sh = shard.T.reshape(NCH, 128, ROWS)           # [256, 128, 512]
        conn8 = np.ascontiguousarray(
            sh.transpose(1, 0, 2).reshape(128, NCH * ROWS)).view(fp8)
        in_maps.append({
            "conn8": conn8,
            "vt2": vt2,
            "boostl": np.ascontiguousarray(b[r * ROWS:(r + 1) * ROWS]),
            "consts16": consts,
        })
    return in_maps


def _run(input_vector, connections, boosting_factors, trace=False, stage=4):
    from concourse import bass_utils

    nc = _build_nc(stage=stage)
    in_maps = _make_in_maps(input_vector, connections, boosting_factors)
    res = bass_utils.run_bass_kernel_spmd(
        nc, in_maps, core_ids=list(range(CORES)), trace=trace,
    )
    out = res.results[0]["out"]
    return (np.ascontiguousarray(out[0]), np.ascontiguousarray(out[1])), res


def kernel(input_vector, connections, boosting_factors):
    (active, masked), _ = _run(input_vector, connections, boosting_factors)
    return active, masked


# revision 39
# speedup vs baseline: 1.0621x; 1.0621x over previous
"""Trainium2 Bass kernel for HTM spatial-pooler overlap + global top-k inhibition.

Problem (nn_HTMModel_19834158973432):
    overlap  = connections @ input_vector          # [4096] = [4096, 32768] @ [32768]
    boosted  = overlap * boosting_factors          # [4096]
    winners  = top_k(boosted, 82)                  # ties broken by lower index
    active   = one_hot(winners)                    # [4096] 0/1 mask
    returns (active, active * boosted)

Strategy (8 NeuronCores, SPMD):
  - connections/input_vector are binary 0/1, so an fp8(e4m3) cast is EXACT.
    Host pre-transposes each core's row shard [512, 32768] into a
    partition-major fp8 layout so the TensorEngine does multiply+accumulate
    in one pass, using DoubleRow fp8 matmuls (2 contractions of K=128 per
    instruction, 2x streaming throughput):
        psum[1, 512] += sum_slot vt2[:, slot, n].T @ conn_pair[:, slot, :]
    DMA traffic is 16 MiB/core (4x less than f32, ~DMA roofline bound);
    conn chunks stream on 3 DMA rings with small first chunks, and ~20
    warmup matmuls keep the PE_HAM clock warm during the fill.
  - boosted = overlap * boost fits fp16 EXACTLY (integers <= 2048); the
    AllGather carries 1 KB/rank of fp16.  A tiny dummy collective at t=0
    absorbs the CC-stream bootstrap during the matvec.
  - Top-82 without sorting: bucket edges are whole boosted values, so
      gp[e] = #{c : boosted[c] >= bmax-127+e}
    is one elementwise pass over a PE-broadcast of the gathered boosted
    vector, split DVE (is_ge halves) / ACT (sign(b-edge+.5) halves), giving
    b82 (boosted value of the 82nd winner) and n_hi (# strictly above);
    the columns with boosted == b82 are then selected by global index rank
    (triangular-matmul prefix across partitions + Kogge-Stone along free).
  - Each core writes the full [2, 4096] output; the host returns core 0's.
"""

import sys

if "/opt/trn_rl_repo" not in sys.path:
    sys.path.insert(0, "/opt/trn_rl_repo")

import numpy as np

C_TOT = 4096          # minicolumns
IN = 32768            # input size
CORES = 8
ROWS = C_TOT // CORES  # 512 rows per core
K_ACT = 82            # active columns per inhibition area

NCH = IN // 128        # 256 i-chunks of 128 (contraction per matmul slot)
NPAIR = NCH // 2       # 128 DoubleRow matmuls

B_MARGIN = 127        # bucket-search window below bmax (in boosted units)
EARLY_CC_WARM = True  # tiny dummy collective at t=0 absorbs CC bootstrap


def _build_nc(stage=4):
    # stage: 1=matvec only, 2=+allgather, 4=full
    from concourse import bacc, mybir, tile

    f32 = mybir.dt.float32
    f16 = mybir.dt.float16
    fp8 = mybir.dt.float8e4
    Alu = mybir.AluOpType
    Act = mybir.ActivationFunctionType
    DR = mybir.MatmulPerfMode.DoubleRow

    nc = bacc.Bacc("TRN2", target_bir_lowering=False, debug=False,
                   enable_asserts=False, num_devices=CORES)

    conn8 = nc.dram_tensor("conn8", [128, NCH * ROWS], fp8, kind="ExternalInput")
    vt2 = nc.dram_tensor("vt2", [128, NCH], fp8, kind="ExternalInput")
    boostl = nc.dram_tensor("boostl", [ROWS], f32, kind="ExternalInput")
    consts16 = nc.dram_tensor("consts16", [128, 256], f16, kind="ExternalInput")
    out = nc.dram_tensor("out", [2, C_TOT], f32, kind="ExternalOutput")

    with tile.TileContext(nc) as tc:
        with (
            tc.tile_pool(name="const", bufs=1) as constp,
            tc.tile_pool(name="cpool", bufs=1) as cpool,
            tc.tile_pool(name="scrp", bufs=2) as scrp,
            tc.tile_pool(name="dramp", bufs=1, space="DRAM") as dramp,
            tc.tile_pool(name="ovp", bufs=1, space="PSUM") as ovp,
            tc.tile_pool(name="warmp", bufs=1, space="PSUM") as warmp,
            tc.tile_pool(name="bcp", bufs=4, space="PSUM") as bcp,
            tc.tile_pool(name="sps", bufs=2, space="PSUM") as sps,
        ):
            # ---- tiny dummy collective first: forces the CC-stream rank
            # handshake to run during the matvec so the real AllGather is
            # not serialized behind a cold bootstrap ----
            if EARLY_CC_WARM and stage >= 2:
                wsrc16 = constp.tile([1, 8], f16, name="wsrc16")
                nc.vector.memset(wsrc16[:], 0.0)
                ccw_in = dramp.tile([8], f16, name="ccw_in")
                nc.gpsimd.dma_start(ccw_in.rearrange("(a f) -> a f", a=1),
                                    wsrc16[:])
                ccw_out = dramp.tile([8 * CORES], f16, name="ccw_out",
                                     addr_space="Shared")
                nc.gpsimd.collective_compute(
                    "AllGather", Alu.bypass,
                    replica_groups=[list(range(CORES))],
                    ins=[ccw_in.opt()], outs=[ccw_out.opt()])

            # ---- matvec input DMAs first: vt2 then the conn chunks.
            # Ascending chunk sizes (in DoubleRow pairs): small first chunks
            # land fast so the MM stream starts early; bandwidth amortizes
            # over the 1 MiB steady-state chunks.
            vt_sb = constp.tile([128, NCH], fp8, name="vt_sb")
            nc.sync.dma_start(vt_sb[:], vt2.ap())
            chunk_pairs = [2, 2, 4, 4] + [8] * 14 + [4]
            assert sum(chunk_pairs) == NPAIR
            cts = []
            off = 0
            engs = [nc.sync, nc.scalar, nc.sync, nc.scalar, nc.gpsimd]
            for k, cp_n in enumerate(chunk_pairs):
                w = cp_n * 2 * ROWS
                ct = cpool.tile([128, w], fp8, name=f"ct_{k}", tag=f"ct{k}")
                # 3 DMA rings: sync/scalar HWDGE take the latency-critical
                # early chunks, gpsimd's SWDGE row adds steady-state bandwidth
                eng = engs[k % len(engs)]
                eng.dma_start(ct[:], conn8.ap()[:, off:off + w])
                cts.append(ct)
                off += w

            # ---- constants (issued on gpsimd, off the critical path) ----
            cs16 = constp.tile([128, 256], f16, name="cs16")
            nc.gpsimd.dma_start(cs16[:], consts16.ap())
            ident16 = cs16[:, 0:128]
            tri16 = cs16[:, 128:256]
            boost_sb = constp.tile([1, ROWS], f32, name="boost_sb")
            nc.gpsimd.dma_start(boost_sb[:], boostl.ap()[None, :])
            ones_row16 = constp.tile([1, 128], f16, name="ones_row16")
            nc.vector.memset(ones_row16[:], 1.0)
            ones_col16 = constp.tile([128, 1], f16, name="ones_col16")
            nc.vector.memset(ones_col16[:], 1.0)
            ramp = constp.tile([128, 1], f32, name="ramp")
            nc.gpsimd.iota(ramp[:], pattern=[[0, 1]], base=0,
                           channel_multiplier=1,
                           allow_small_or_imprecise_dtypes=True)

            # ---- PE warmup: ~20 throwaway matmuls during the DMA fill keep
            # the PE_HAM activity window busy so the real MM stream runs at
            # 2.4 GHz (warm) instead of 1.2 GHz (cold). ----
            wrow16 = constp.tile([1, 512], f16, name="wrow16")
            nc.vector.memset(wrow16[:], 0.0)
            warm_ps = warmp.tile([128, 512], f32, name="warm_ps")
            for w in range(14):
                nc.tensor.matmul(warm_ps[:], lhsT=ones_row16[:], rhs=wrow16[:],
                                 start=True, stop=True)

            # ---- matvec: 128 DoubleRow fp8 matmuls accumulate into PSUM ----
            ov_ps = ovp.tile([1, ROWS], f32, name="ov_ps", tag="ov")
            vt_pairs = vt_sb.rearrange("p (two n) -> p two n", two=2)
            pr = 0
            for k, cp_n in enumerate(chunk_pairs):
                ctp = cts[k].rearrange("p (j two n) -> p j two n", j=cp_n,
                                       two=2)
                for j in range(cp_n):
                    nc.tensor.matmul(
                        ov_ps[:],
                        lhsT=vt_pairs[:, :, pr:pr + 1],
                        rhs=ctp[:, j],
                        start=(pr == 0), stop=(pr == NPAIR - 1),
                        perf_mode=DR,
                    )
                    pr += 1
            assert pr == NPAIR

            if stage <= 1:
                nc.sync.dma_start(out.ap()[0][0:ROWS][None, :], ov_ps[:])
                nc.compile()
                return nc

            # ---- boosted (fp16-exact) -> AllGather 1 KB/rank ----
            bl16 = constp.tile([1, ROWS], f16, name="bl16")
            nc.vector.tensor_tensor(bl16[:], ov_ps[:], boost_sb[:], Alu.mult)
            cc_in = dramp.tile([ROWS], f16, name="cc_in")
            cc_out = dramp.tile([C_TOT], f16, name="cc_out",
                                addr_space="Shared")
            nc.sync.dma_start(cc_in.rearrange("(a f) -> a f", a=1), bl16[:])
            nc.gpsimd.collective_compute(
                "AllGather", Alu.bypass,
                replica_groups=[list(range(CORES))],
                ins=[cc_in.opt()], outs=[cc_out.opt()])
            # keep the PE_HAM busy across the AllGather wait so the tail
            # matmuls run warm (2.4 GHz)
            if stage >= 3:
                for w in range(26):
                    nc.tensor.matmul(warm_ps[:], lhsT=ones_row16[:],
                                     rhs=wrow16[:], start=True, stop=True)

            if stage == 2:
                t16 = constp.tile([128, 32], f16, name="t16")
                nc.sync.dma_start(t16[:],
                                  cc_out.rearrange("(p f) -> p f", p=128))
                t32 = constp.tile([128, 32], f32, name="t32")
                nc.vector.tensor_copy(t32[:], t16[:])
                nc.sync.dma_start(
                    out.ap()[0].rearrange("(p f) -> p f", p=128), t32[:])
                nc.compile()
                return nc

            # ---- gathered boosted in two layouts (concurrent DMAs) ----
            b32_16 = constp.tile([128, 32], f16, name="b32_16")
            nc.sync.dma_start(b32_16[:], cc_out.rearrange("(p f) -> p f", p=128))
            brow16 = constp.tile([1, C_TOT], f16, name="brow16")
            nc.scalar.dma_start(brow16[:], cc_out.rearrange("(a f) -> a f", a=1))

            # ---- bmax (cross-partition max via fp16 PE transpose) ----
            bm16 = constp.tile([128, 1], f16, name="bm16")
            with nc.allow_low_precision(reason="max is fp16-exact"):
                nc.vector.reduce_max(bm16[:], b32_16[:],
                                     axis=mybir.AxisListType.X)
            bm_row = sps.tile([1, 128], f32, name="bm_row", tag="sps")
            nc.tensor.matmul(bm_row[:], lhsT=bm16[:], rhs=ident16,
                             start=True, stop=True)
            bmax16 = constp.tile([1, 1], f16, name="bmax16")
            with nc.allow_low_precision(reason="max is fp16-exact"):
                nc.vector.reduce_max(bmax16[:], bm_row[:],
                                     axis=mybir.AxisListType.X)
            bmax_ps = sps.tile([128, 1], f32, name="bmax_ps", tag="sps")
            nc.tensor.matmul(bmax_ps[:], lhsT=ones_row16[:], rhs=bmax16[:],
                             start=True, stop=True)
            # edges[p] = bmax - 127 + p ; nedge[p] = -edges[p] + 0.5
            b127 = constp.tile([128, 1], f32, name="b127")
            nc.vector.tensor_scalar(
                out=b127[:], in0=bmax_ps[:], scalar1=-float(B_MARGIN),
                scalar2=None, op0=Alu.add)
            edges = constp.tile([128, 1], f32, name="edges")
            nc.vector.tensor_tensor(edges[:], ramp[:], b127[:], Alu.add)
            bneg = constp.tile([128, 1], f32, name="bneg")
            nc.vector.tensor_scalar(
                out=bneg[:], in0=bmax_ps[:], scalar1=-1.0,
                scalar2=float(B_MARGIN) + 0.5, op0=Alu.mult, op1=Alu.add)
            nedge = constp.tile([128, 1], f32, name="nedge")
            nc.vector.tensor_scalar(
                out=nedge[:], in0=ramp[:], scalar1=-1.0, scalar2=bneg[:],
                op0=Alu.mult, op1=Alu.add)
            boosted32 = constp.tile([128, 32], f32, name="boosted32")
            nc.vector.tensor_copy(boosted32[:], b32_16[:])

            # ---- PE-broadcast boosted to 128 partitions (quarters), then
            # count cnt_ge(edge_p): quarters 0,1 on DVE is_ge, 2,3 on ACT
            # via sum of sign(b - edge + 0.5) ----
            # eighth-sized broadcast tiles: 8 PE matmuls (N=512) feed the
            # two count engines round-robin (evens on DVE is_ge, odds on ACT
            # Sign); bufs=4 PSUM quarters avoid reuse stalls
            g_all = constp.tile([128, 8], f32, name="g_all")
            for h in (0, 1, 2, 3, 4, 5, 6, 7):
                ps = bcp.tile([128, 512], f32, name=f"kbc{h}", tag="kbc")
                nc.tensor.matmul(
                    ps[:], lhsT=ones_row16[:],
                    rhs=brow16[:, h * 512:(h + 1) * 512],
                    start=True, stop=True)
                scr = scrp.tile([128, 512], f16, name=f"cmp{h}",
                                tag="cmp", bufs=4)
                if h % 2 == 0:
                    nc.vector.tensor_scalar(
                        out=scr[:], in0=ps[:], scalar1=edges[:], scalar2=None,
                        op0=Alu.is_ge, op1=Alu.add,
                        accum_out=g_all[:, h:h + 1])
                else:
                    nc.scalar.activation(scr[:], ps[:], Act.Sign,
                                         bias=nedge[:], scale=1.0,
                                         accum_out=g_all[:, h:h + 1])
            gview = g_all.rearrange("p (f two) -> p f two", two=2)
            gp01 = constp.tile([128, 1], f32, name="gp01")
            nc.vector.reduce_sum(gp01[:], gview[:, :, 0:1],
                                 axis=mybir.AxisListType.XY)
            gp23 = constp.tile([128, 1], f32, name="gp23")
            nc.vector.reduce_sum(gp23[:], gview[:, :, 1:2],
                                 axis=mybir.AxisListType.XY)
            # sign sums: g = 2*cnt - 512 per eighth -> cnt_odds = g/2 + 1024
            gp23s = constp.tile([128, 1], f32, name="gp23s")
            nc.vector.tensor_scalar(
                out=gp23s[:], in0=gp23[:], scalar1=0.5, scalar2=1024.0,
                op0=Alu.mult, op1=Alu.add)
            gp = constp.tile([128, 1], f32, name="gp")
            nc.vector.tensor_tensor(gp[:], gp01[:], gp23s[:], Alu.add)

            # cnt = #edges with cnt_ge >= 82  ->  b82 = bmax - 128 + cnt
            sel16 = constp.tile([128, 1], f16, name="sel16")
            nc.vector.tensor_scalar(
                out=sel16[:], in0=gp[:], scalar1=float(K_ACT), scalar2=None,
                op0=Alu.is_ge)
            cnt_ps = sps.tile([1, 1], f32, name="cnt_ps", tag="sps")
            nc.tensor.matmul(cnt_ps[:], lhsT=sel16[:], rhs=ones_col16[:],
                             start=True, stop=True)
            # n_hi = max over edges of gp*(gp<82)  (= cnt above b82's bucket)
            lt = constp.tile([128, 1], f32, name="lt")
            nc.vector.tensor_scalar(
                out=lt[:], in0=gp[:], scalar1=float(K_ACT), scalar2=None,
                op0=Alu.is_lt)
            gpm16 = constp.tile([128, 1], f16, name="gpm16")
            nc.vector.tensor_tensor(gpm16[:], gp[:], lt[:], Alu.mult)
            nhi_row = sps.tile([1, 128], f32, name="nhi_row", tag="sps")
            nc.tensor.matmul(nhi_row[:], lhsT=gpm16[:], rhs=ident16,
                             start=True, stop=True)
            nhi1 = constp.tile([1, 1], f32, name="nhi1")
            nc.vector.reduce_max(nhi1[:], nhi_row[:], axis=mybir.AxisListType.X)

            # pair16 = [b82, m]: b82 = (bmax - 128) + cnt ; m = 82 - n_hi
            bshift = constp.tile([1, 1], f32, name="bshift")
            nc.vector.tensor_scalar(
                out=bshift[:], in0=bmax16[:], scalar1=-128.0, scalar2=None,
                op0=Alu.add)
            pair16 = constp.tile([1, 2], f16, name="pair16")
            nc.vector.tensor_tensor(pair16[:, 0:1], cnt_ps[:], bshift[:],
                                    Alu.add)
            nc.vector.tensor_scalar(
                out=pair16[:, 1:2], in0=nhi1[:], scalar1=-1.0,
                scalar2=float(K_ACT), op0=Alu.mult, op1=Alu.add)
            pair_ps = sps.tile([128, 2], f32, name="pair_ps", tag="sps")
            nc.tensor.matmul(pair_ps[:], lhsT=ones_row16[:], rhs=pair16[:],
                             start=True, stop=True)
            pair_bc = constp.tile([128, 2], f32, name="pair_bc")
            nc.scalar.activation(pair_bc[:], pair_ps[:], Act.Copy)

            # ---- masks + global rank of equals ----
            gt32 = constp.tile([128, 32], f32, name="gt32")
            nc.vector.tensor_scalar(
                out=gt32[:], in0=boosted32[:], scalar1=pair_bc[:, 0:1],
                scalar2=None, op0=Alu.is_gt)
            eq32 = constp.tile([128, 32], f32, name="eq32")
            nc.vector.tensor_scalar(
                out=eq32[:], in0=boosted32[:], scalar1=pair_bc[:, 0:1],
                scalar2=None, op0=Alu.is_equal)
            # exclusive prefix count of eq across global c order:
            # per-partition totals -> strictly-lower-triangular matmul.
            # s16 summed on ACT so it overlaps the Kogge-Stone on DVE.
            s16 = constp.tile([128, 1], f16, name="s16")
            s_scr = constp.tile([128, 32], f16, name="s_scr")
            with nc.allow_low_precision(reason="sum of <=32 ones, fp16-exact"):
                nc.scalar.activation(s_scr[:], eq32[:], Act.Copy,
                                     accum_out=s16[:])
            P_ps = sps.tile([128, 1], f32, name="P_ps", tag="sps")
            nc.tensor.matmul(P_ps[:], lhsT=tri16, rhs=s16[:],
                             start=True, stop=True)
            P_sb = constp.tile([128, 1], f32, name="P_sb")
            nc.scalar.activation(P_sb[:], P_ps[:], Act.Copy)
            # within-partition inclusive prefix via Kogge-Stone on padded rows
            pada = constp.tile([128, 48], f32, name="pada")
            padb = constp.tile([128, 48], f32, name="padb")
            nc.vector.memset(pada[:, 0:16], 0.0)
            nc.vector.memset(padb[:, 0:16], 0.0)
            nc.vector.tensor_copy(pada[:, 16:48], eq32[:])
            src, dst = pada, padb
            for sh in (1, 2, 4, 8, 16):
                # pads [0:16] stay zero in both buffers (never written)
                nc.vector.tensor_tensor(
                    dst[:, 16:48], src[:, 16:48], src[:, 16 - sh:48 - sh],
                    Alu.add)
                src, dst = dst, src
            incl = src  # inclusive prefix in [:, 16:48]
            # rank among eq in global c order: local excl prefix < m - P
            rank32 = constp.tile([128, 32], f32, name="rank32")
            nc.vector.tensor_tensor(rank32[:], incl[:, 16:48], eq32[:],
                                    Alu.subtract)
            mP = constp.tile([128, 1], f32, name="mP")
            nc.vector.tensor_scalar(
                out=mP[:], in0=P_sb[:], scalar1=-1.0,
                scalar2=pair_bc[:, 1:2], op0=Alu.mult, op1=Alu.add)
            # tie-selected = eq & (rank < m)
            tie32 = constp.tile([128, 32], f32, name="tie32")
            nc.vector.tensor_scalar(
                out=tie32[:], in0=rank32[:], scalar1=mP[:],
                scalar2=None, op0=Alu.is_lt)
            nc.vector.tensor_tensor(tie32[:], tie32[:], eq32[:], Alu.mult)

            # active|masked side by side in one tile -> single output DMA
            am = constp.tile([128, 64], f32, name="am")
            active32 = am[:, 0:32]
            nc.vector.tensor_tensor(active32, gt32[:], tie32[:], Alu.add)
            nc.vector.tensor_tensor(am[:, 32:64], active32, boosted32[:],
                                    Alu.mult)
            nc.sync.dma_start(
                out.ap().rearrange("r (p f) -> p r f", p=128),
                am.rearrange("p (r f) -> p r f", r=2))

    nc.compile()
    return nc


FP8_ONE = 0x38  # ml_dtypes.float8_e4m3(1.0).view(uint8)


def _make_in_maps(input_vector, connections, boosting_factors):
    import ml_dtypes

    fp8 = ml_dtypes.float8_e4m3
    v = np.asarray(input_vector, dtype=np.float32)
    c = np.asarray(connections, dtype=np.float32)
    b = np.ascontiguousarray(np.asarray(boosting_factors, dtype=np.float32))

    # consts16: [ident | strictly-lower-tri (tri[q,p]=1 iff q<p)]
    consts = np.zeros((128, 256), dtype=np.float16)
    consts[:, 0:128] = np.eye(128, dtype=np.float16)
    consts[:, 128:256] = np.triu(np.ones((128, 128), dtype=np.float16), k=1)

    # vt2[p, n] = v[(2n)*128 + p] for n<128, v[(2n+1)*128 + p] for n>=128
    vt = (v.reshape(NCH, 128).T != 0) * np.uint8(FP8_ONE)  # [128, 256]
    vt2 = np.ascontiguousarray(
        np.concatenate([vt[:, 0::2], vt[:, 1::2]], axis=1)).view(fp8)

    c8 = (c != 0) * np.uint8(FP8_ONE)  # [4096, 32768] uint8
    in_maps = []
    for r in range(CORES):
        # conn8[p, n*512 + j] = conn[r*512 + j, n*128 + p]
        shard = c8[r * ROWS:(r + 1) * ROWS]            # [512, 32768]
        sh = shard.T.reshape(NCH, 128, ROWS)           # [256, 128, 512]
        conn8 = np.ascontiguousarray(
            sh.transpose(1, 0, 2).reshape(128, NCH * ROWS)).view(fp8)
        in_maps.append({
            "conn8": conn8,
            "vt2": vt2,
            "boostl": np.ascontiguousarray(b[r * ROWS:(r + 1) * ROWS]),
            "consts16": consts,
        })
    return in_maps


def _run(input_vector, connections, boosting_factors, trace=False, stage=4):
    from concourse import bass_utils

    nc = _build_nc(stage=stage)
    in_maps = _make_in_maps(input_vector, connections, boosting_factors)
    res = bass_utils.run_bass_kernel_spmd(
        nc, in_maps, core_ids=list(range(CORES)), trace=trace,
    )
    out = res.results[0]["out"]
    return (np.ascontiguousarray(out[0]), np.ascontiguousarray(out[1])), res


def kernel(input_vector, connections, boosting_factors):
    (active, masked), _ = _run(input_vector, connections, boosting_factors)
    return active, masked
